# revision 7
# baseline (speedup 1.0000x reference)
"""NT-Xent (SimCLR) contrastive loss on 8 Trainium2 NeuronCores.

Math: with x = row-normalized representation [8192, 256], tau = 0.5,
  sim = x @ x.T
  loss = (1/8192) * sum_i [ ln(sum_{j != i} exp(sim[i,j]/tau)) - sim[i, pos(i)]/tau ]
where pos(i) = (i + 4096) mod 8192.

Sharding: data-parallel over rows. Core c owns rows [c*1024, (c+1)*1024).
Each core receives the full representation (to build the transposed,
normalized key matrix xT in bf16), plus its own row slab and the partner
rows (i+4096 mod 8192) as separate inputs, so the single SPMD NEFF needs
no per-core control flow. Each core computes its [1024, 8192] similarity
slab with bf16 matmuls, exp+row-sum on the scalar engine (accum_out),
and the positive/diagonal terms via fp32 row-major dot products. The
per-row losses [128, 8] are DMA'd out; the host sums the 8 partials.

xT is stored as 16 column-chunk tiles of [128, 512] per K-half so phase-2
matmuls on chunk j can start as soon as its 4 source row-tiles are
transposed, overlapping with the rest of phase 1.
"""

import numpy as np
import ml_dtypes

import concourse.bacc as bacc
import concourse.bass as bass
import concourse.tile as tile
from concourse import mybir
from concourse.bass_utils import run_bass_kernel_spmd

N2 = 8192            # total rows (2N)
D = 256              # feature dim
NCORES = 8
ROWS = N2 // NCORES  # 1024 rows per core
N = N2 // 2          # positive-pair offset
P = 128              # SBUF partitions
KC = D // P          # 2 contraction chunks of 128
T_FULL = N2 // P     # 64 row tiles of the full matrix
T_SLAB = ROWS // P   # 8 row tiles of the slab
CCH = 512            # xT column-chunk width (= max matmul moving free)
NJ = N2 // CCH       # 16 column chunks
ACH = 1024           # activation chunk width (2 PSUM banks)
NJ2 = N2 // ACH      # 8 exp/rowsum chunks

F32 = mybir.dt.float32
BF16 = mybir.dt.bfloat16
AF = mybir.ActivationFunctionType
ALU = mybir.AluOpType


def _build_kernel(tc: tile.TileContext, out_ap, rep, slab, partner, ident_in):
    nc = tc.nc
    with (
        tc.tile_pool(name="const", bufs=1) as const,
        tc.tile_pool(name="persist", bufs=1) as persist,
        tc.tile_pool(name="work", bufs=4) as work,
        tc.tile_pool(name="small", bufs=4) as small,
        tc.tile_pool(name="exps", bufs=4) as exps,
        tc.tile_pool(name="pst", bufs=2, space="PSUM") as pst,
        tc.tile_pool(name="psmm", bufs=3, space="PSUM") as psmm,
    ):
        ident = const.tile([P, P], BF16, name="ident")
        nc.sync.dma_start(out=ident, in_=ident_in)
        ln2 = const.tile([P, 1], F32, name="ln2")
        nc.vector.memset(ln2, 0.6931471805599453)

        # persistent state
        xTc = [[persist.tile([P, CCH], BF16, tag=f"xT{k}_{j}", name=f"xT{k}_{j}")
                for j in range(NJ)] for k in range(KC)]
        sT = [persist.tile([P, ROWS], BF16, tag=f"sT{k}", name=f"sT{k}")
              for k in range(KC)]
        rsums = [persist.tile([P, NJ2], F32, tag=f"rs{m}", name=f"rs{m}")
                 for m in range(T_SLAB)]
        d_all = persist.tile([P, T_SLAB], F32, tag="d_all", name="d_all")
        pos2 = persist.tile([P, T_SLAB], F32, tag="pos2", name="pos2")
        sxm = persist.tile([P, T_SLAB], F32, tag="sxm", name="sxm")
        lossm = persist.tile([P, T_SLAB], F32, tag="lossm", name="lossm")

        def load_norm(src, t, out_dt, tag, logbias=None):
            """DMA row-tile t of src; return (x * exp(-0.5*ln||x||^2 + logbias),
            raw x, inv scale). logbias=ln(2) yields rows scaled by 2/||row||.
            rsqrt is computed as exp(-0.5*ln(ssq)) -- the InstReciprocal and
            tensor_scalar-with-AP-scalar paths abort on this runtime.
            """
            x = work.tile([P, D], F32, tag=f"ld_{tag}", name=f"ld_{tag}")
            nc.sync.dma_start(out=x, in_=src[t * P:(t + 1) * P, :])
            sq = work.tile([P, D], F32, tag="sq", name="sq")
            nc.vector.tensor_mul(sq, x, x)
            ssq = small.tile([P, 1], F32, tag="ssq", name="ssq")
            nc.vector.reduce_sum(ssq, sq, axis=mybir.AxisListType.X)
            lssq = small.tile([P, 1], F32, tag="lssq", name="lssq")
            nc.scalar.activation(lssq, ssq, AF.Ln)
            inv = small.tile([P, 1], F32, tag="inv", name="inv")
            nc.scalar.activation(inv, lssq, AF.Exp, scale=-0.5,
                                 bias=0.0 if logbias is None else logbias)
            xn = work.tile([P, D], out_dt, tag=f"xn_{tag}", name=f"xn_{tag}")
            nc.scalar.activation(xn, x, AF.Copy, scale=inv)
            return xn, x, inv

        def transpose_tile(xb, put):
            """xb [128, 256] bf16; put(k, psum_tile) stores the k-th half."""
            for k in range(KC):
                pt = pst.tile([P, P], BF16, tag="pt", name="pt")
                nc.tensor.transpose(pt, xb[:, k * P:(k + 1) * P], ident)
                put(k, pt)

        # phase 1a: slab rows -> sT (bf16 queries) + d_i; partner -> pos2_i
        for t in range(T_SLAB):
            xs, xraw, inv = load_norm(slab, t, F32, "slab")
            xsb = work.tile([P, D], BF16, tag="xsb", name="xsb")
            nc.scalar.activation(xsb, xraw, AF.Copy, scale=inv)
            transpose_tile(
                xsb, lambda k, pt, t=t: nc.vector.tensor_copy(
                    sT[k][:, t * P:(t + 1) * P], pt))
            sq2 = work.tile([P, D], F32, tag="sq2", name="sq2")
            nc.vector.tensor_mul(sq2, xs, xs)
            nc.vector.reduce_sum(d_all[:, t:t + 1], sq2,
                                 axis=mybir.AxisListType.X)
            xp, _, _ = load_norm(partner, t, F32, "part", logbias=ln2)
            sq3 = work.tile([P, D], F32, tag="sq3", name="sq3")
            nc.vector.tensor_mul(sq3, xs, xp)
            nc.vector.reduce_sum(pos2[:, t:t + 1], sq3,
                                 axis=mybir.AxisListType.X)

        # phase 1b: full rep -> normalized, transposed key chunks xTc (bf16)
        for t in range(T_FULL):
            xb, _, _ = load_norm(rep, t, BF16, "full")
            j, off = divmod(t * P, CCH)
            transpose_tile(
                xb, lambda k, pt, j=j, off=off: nc.vector.tensor_copy(
                    xTc[k][j][:, off:off + P], pt))

        # phase 2: similarity slab in [128, 1024] chunks; exp + row sums.
        # j2-outer so chunk j2 only needs xTc[:][2*j2:2*j2+2] (overlaps ph1b).
        for j2 in range(NJ2):
            for m in range(T_SLAB):
                ps = psmm.tile([P, ACH], F32, tag="ps", name="ps")
                for half in range(2):
                    jj = 2 * j2 + half
                    for k in range(KC):
                        nc.tensor.matmul(
                            ps[:, half * CCH:(half + 1) * CCH],
                            sT[k][:, m * P:(m + 1) * P],
                            xTc[k][jj],
                            start=(k == 0), stop=(k == KC - 1))
                esc = exps.tile([P, ACH], BF16, tag="esc", name="esc")
                nc.scalar.activation(esc, ps, AF.Exp, scale=2.0,
                                     accum_out=rsums[m][:, j2:j2 + 1])

        # tails: S_m - exp(2 d_m), then one Ln + subtract over all columns
        for m in range(T_SLAB):
            S = small.tile([P, 1], F32, tag="S", name="S")
            nc.vector.reduce_sum(S, rsums[m], axis=mybir.AxisListType.X)
            ed = small.tile([P, 1], F32, tag="ed", name="ed")
            nc.scalar.activation(ed, d_all[:, m:m + 1], AF.Exp, scale=2.0)
            nc.vector.tensor_sub(sxm[:, m:m + 1], S, ed)
        nc.scalar.activation(lossm, sxm, AF.Ln)
        nc.vector.tensor_sub(lossm, lossm, pos2)
        nc.sync.dma_start(out=out_ap, in_=lossm)


def build_nc():
    nc = bacc.Bacc("TRN2", target_bir_lowering=False, debug=False,
                   num_devices=NCORES)
    rep = nc.dram_tensor("rep", [N2, D], F32, kind="ExternalInput").ap()
    slab = nc.dram_tensor("slab", [ROWS, D], F32, kind="ExternalInput").ap()
    partner = nc.dram_tensor("partner", [ROWS, D], F32,
                             kind="ExternalInput").ap()
    ident_in = nc.dram_tensor("ident", [P, P], BF16,
                              kind="ExternalInput").ap()
    out = nc.dram_tensor("out", [P, T_SLAB], F32, kind="ExternalOutput").ap()
    with tile.TileContext(nc) as tc:
        _build_kernel(tc, out, rep, slab, partner, ident_in)
    nc.compile()
    return nc


_NC = None
LAST_RESULTS = None
_IDENT = np.eye(P, dtype=np.float32).astype(ml_dtypes.bfloat16)


def _make_in_maps(rep: np.ndarray):
    in_maps = []
    for c in range(NCORES):
        r0 = c * ROWS
        slab = np.ascontiguousarray(rep[r0:r0 + ROWS])
        pidx = (np.arange(r0, r0 + ROWS) + N) % N2
        partner = np.ascontiguousarray(rep[pidx])
        in_maps.append({"rep": rep, "slab": slab, "partner": partner,
                        "ident": _IDENT})
    return in_maps


def kernel(representation: np.ndarray, **run_kwargs) -> np.ndarray:
    global _NC, LAST_RESULTS
    rep = np.ascontiguousarray(np.asarray(representation), dtype=np.float32)
    assert rep.shape == (N2, D)
    if _NC is None:
        _NC = build_nc()
    res = run_bass_kernel_spmd(_NC, _make_in_maps(rep),
                               core_ids=list(range(NCORES)), **run_kwargs)
    LAST_RESULTS = res
    total = 0.0
    for r in res.results:
        total += float(r["out"].astype(np.float64).sum())
    return np.asarray(np.float32(total / N2))
